# revision 18
# baseline (speedup 1.0000x reference)
"""Trainium2 Bass kernel for nn_MoE_for_Pruning (moe_routing).

Self-contained: kernel(**inputs) takes full unsharded inputs, shards across
8 NeuronCores, runs one SPMD Bass/Tile kernel, gathers full outputs.
"""
import math

import numpy as np

import concourse.bacc as bacc
import concourse.bass as bass
import concourse.mybir as mybir
import concourse.tile as tile
from concourse import bass_utils
from concourse.masks import make_identity

dt = mybir.dt
Alu = mybir.AluOpType
Act = mybir.ActivationFunctionType
Axis = mybir.AxisListType

NCORES = 8
N = 512000
D = 64
B = 32
NEDGE = 1000000
NDIFF = N // 2                  # 256000 diff nodes (n >= NDIFF)
NSH = N // NCORES               # 64000 nodes/core (2 blocks of 32000)
HBLK = NSH // 2                 # 32000
ESH = NEDGE // NCORES           # 125000 edges/core
EP = 1000                       # edges per partition ([125, 1000])
CAND = 24                       # local candidates per partition
CPAD = 32                       # padded candidate slots per partition
SLAB = 40960                    # allgather slab (f32 elements)
COS_OFF = 0                     # [0, 32000) cos shard
VAL_OFF = 32000                 # [32000, 36000) cand vals, p*32+r
OBJ_OFF = 36000                 # [36000, 40000) cand objs
SEARCH_ITERS = 30
NEG = -1.0e30
SENT = -3.0e30

# reference's layer-dependent top-k schedule (python floats, same as ref)
K_SOURCE = 100; K_MIN = 50; K_MAX = 1000; L_INFL = 4; A_SLOPE = 1.0


def compute_topk(l):
    if l < L_INFL:
        s = 1.0 / (1.0 + math.exp(-A_SLOPE * (l - L_INFL / 2)))
        return int(K_SOURCE + (K_MAX - K_SOURCE) * s)
    s = 1.0 / (1.0 + math.exp(-A_SLOPE * (l - 3 * L_INFL / 2)))
    return int(K_MIN + (K_MAX - K_MIN) * (1.0 - s))


def build_kernel(debug=False):
    nc = bacc.Bacc("TRN2", target_bir_lowering=False, debug=False,
                   num_devices=NCORES)

    def inp(name, shape, d=dt.float32):
        return nc.dram_tensor(name, list(shape), d, kind="ExternalInput")

    def outp(name, shape, d=dt.float32):
        return nc.dram_tensor(name, list(shape), d, kind="ExternalOutput")

    # ---- inputs (per core) ----
    hid_nd = inp("hid_nd", [HBLK, D]); hid_df = inp("hid_df", [HBLK, D])
    h0_nd = inp("h0_nd", [HBLK, D]); h0_df = inp("h0_df", [HBLK, D])
    sc_nd = inp("sc_nd", [HBLK, 1]); sc_df = inp("sc_df", [HBLK, 1])
    sc_diff_full = inp("sc_diff_full", [NDIFF, 1])
    noise_sh = inp("noise_sh", [NSH, 3])
    alpha_sh = inp("alpha_sh", [ESH, 1])
    obj_sh = inp("obj_sh", [ESH, 1])          # f32 (exact ints)
    hidden_q = inp("hidden_q", [B, D])
    q_rel = inp("q_rel", [B, 1], dt.int32)
    rel_embed = inp("rel_embed", [401, D])
    mlp_w1 = inp("mlp_w1", [2 * D, D]); mlp_b1 = inp("mlp_b1", [D, 1])
    mlp_w2 = inp("mlp_w2", [D, D]); mlp_b2 = inp("mlp_b2", [D, 1])
    expert_emb = inp("expert_emb", [3, D]); w_n = inp("w_n", [D, 1])
    k_in = inp("k_in", [1, 1])                # K_l as f32
    cbase = inp("cbase", [1, 1])              # core id as f32

    # ---- outputs ----
    hidc_nd = outp("hidc_nd", [HBLK, D]); hidc_df = outp("hidc_df", [HBLK, D])
    h0c_nd = outp("h0c_nd", [HBLK, D]); h0c_df = outp("h0c_df", [HBLK, D])
    scc_nd = outp("scc_nd", [HBLK, 1]); scc_df = outp("scc_df", [HBLK, 1])
    limp = outp("limp", [1, 1])
    if debug:
        dbg_t = outp("dbg_t", [128, 4])       # t0 | t1 | t2 | counts
        dbg_w = outp("dbg_w", [1, 8])
        dbg_wm = outp("dbg_wm", [125, 256])
        dbg_cos = outp("dbg_cos", [125, 256])

    from contextlib import ExitStack
    with tile.TileContext(nc) as tc, ExitStack() as ctx:
        sb = ctx.enter_context(tc.tile_pool(name="sb", bufs=1))   # persistents
        st = ctx.enter_context(tc.tile_pool(name="st", bufs=3))   # streamed
        ps = ctx.enter_context(tc.tile_pool(name="ps", bufs=2, space="PSUM"))
        dr = ctx.enter_context(tc.tile_pool(name="dr", bufs=1, space="DRAM"))

        f32 = dt.float32

        # ================= stage 0: constants & tiny MLP =================
        id128 = sb.tile([128, 128], f32)
        make_identity(nc, id128[:])
        ones_1_125 = sb.tile([1, 125], f32); nc.vector.memset(ones_1_125[:], 1.0)
        ones_1_128 = sb.tile([1, 128], f32); nc.vector.memset(ones_1_128[:], 1.0)
        ones125sq = sb.tile([125, 125], f32); nc.vector.memset(ones125sq[:], 1.0)
        ones125_1 = sb.tile([125, 1], f32); nc.vector.memset(ones125_1[:], 1.0)
        onecol = sb.tile([125, 1], f32); nc.vector.memset(onecol[:], 1.0)

        # merge matrix M128[P, m] = (P % 32 == m % 32)
        m128i = st.tile([128, 128], dt.int32, tag="m128s", bufs=1, name="m128i")
        nc.gpsimd.iota(m128i[:], pattern=[[-1, 128]], base=0, channel_multiplier=1)
        m128a = st.tile([128, 128], dt.int32, tag="m128s2", bufs=1, name="m128a")
        nc.vector.tensor_scalar(out=m128a[:], in0=m128i[:], scalar1=31,
                                scalar2=None, op0=Alu.bitwise_and)
        M128 = sb.tile([128, 128], f32)
        nc.vector.tensor_scalar(out=M128[:], in0=m128a[:], scalar1=0,
                                scalar2=None, op0=Alu.is_equal)

        def bcast_col(src11, parts=125, name=None):
            """[1,1] -> [parts,1] via PE."""
            p = ps.tile([parts, 1], f32, tag="pp")
            nc.tensor.matmul(out=p[:], lhsT=ones_1_125[:, :parts], rhs=src11,
                             start=True, stop=True)
            t = sb.tile([parts, 1], f32, name=name)
            nc.vector.tensor_copy(t[:], p[:])
            return t

        def bcast_row(src_row, width, parts=125, name=None):
            """[1,W] -> [parts,W] via PE (W<=512 per matmul chunk)."""
            t = sb.tile([parts, width], f32, name=name)
            for o in range(0, width, 512):
                w = min(512, width - o)
                p = ps.tile([parts, 512], f32, tag="pp")
                nc.tensor.matmul(out=p[:, :w], lhsT=ones_1_125[:, :parts],
                                 rhs=src_row[:, o:o + w], start=True, stop=True)
                nc.vector.tensor_copy(t[:, o:o + w], p[:, :w])
            return t

        def transpose_pe(in_ap, rows, cols, name=None):
            """[rows, cols] sbuf -> [cols, rows] sbuf via PE."""
            p = ps.tile([cols, rows], f32, tag="pp")
            nc.tensor.transpose(p[:], in_ap, id128[:rows, :rows])
            t = sb.tile([cols, rows], f32, name=name)
            nc.vector.tensor_copy(t[:], p[:])
            return t

        # hidden_q / q_rel_e
        hq = sb.tile([B, D], f32)
        nc.sync.dma_start(hq[:], hidden_q[:])
        qrl = sb.tile([B, 1], dt.int32)
        nc.sync.dma_start(qrl[:], q_rel[:])
        qre = sb.tile([B, D], f32)
        nc.gpsimd.indirect_dma_start(
            out=qre[:], out_offset=None, in_=rel_embed[:],
            in_offset=bass.IndirectOffsetOnAxis(ap=qrl[:, :1], axis=0))
        hqT = transpose_pe(hq[:], B, D, name="hqT")
        qreT = transpose_pe(qre[:], B, D, name="qreT")
        ctxT_in = sb.tile([2 * D, B], f32)
        nc.vector.tensor_copy(ctxT_in[0:D, :], hqT[:])
        nc.vector.tensor_copy(ctxT_in[D:2 * D, :], qreT[:])

        w1sb = sb.tile([2 * D, D], f32); nc.sync.dma_start(w1sb[:], mlp_w1[:])
        w2sb = sb.tile([D, D], f32); nc.sync.dma_start(w2sb[:], mlp_w2[:])
        b1sb = sb.tile([D, 1], f32); nc.sync.dma_start(b1sb[:], mlp_b1[:])
        b2sb = sb.tile([D, 1], f32); nc.sync.dma_start(b2sb[:], mlp_b2[:])
        eesb = sb.tile([3, D], f32); nc.sync.dma_start(eesb[:], expert_emb[:])
        wnsb = sb.tile([D, 1], f32); nc.sync.dma_start(wnsb[:], w_n[:])
        ksb = sb.tile([1, 1], f32); nc.sync.dma_start(ksb[:], k_in[:])
        cbsb = sb.tile([1, 1], f32); nc.sync.dma_start(cbsb[:], cbase[:])

        h1p = ps.tile([D, B], f32, tag="pp")
        nc.tensor.matmul(out=h1p[:], lhsT=w1sb[:], rhs=ctxT_in[:], start=True, stop=True)
        h1T = sb.tile([D, B], f32)
        nc.scalar.activation(h1T[:], h1p[:], Act.Relu, bias=b1sb[:])
        c2p = ps.tile([D, B], f32, tag="pp")
        nc.tensor.matmul(out=c2p[:], lhsT=w2sb[:], rhs=h1T[:], start=True, stop=True)
        ctxT = sb.tile([D, B], f32)
        nc.vector.tensor_tensor(out=ctxT[:], in0=c2p[:],
                                in1=b2sb[:].to_broadcast([D, B]), op=Alu.add)
        eeT = transpose_pe(eesb[:], 3, D, name="eeT")
        escp = ps.tile([3, B], f32, tag="pp")
        nc.tensor.matmul(out=escp[:], lhsT=eeT[:], rhs=ctxT[:], start=True, stop=True)
        escT = sb.tile([3, B], f32)
        nc.vector.tensor_copy(escT[:], escp[:])
        nscp = ps.tile([1, B], f32, tag="pp")
        nc.tensor.matmul(out=nscp[:], lhsT=wnsb[:], rhs=ctxT[:], start=True, stop=True)
        nscT = sb.tile([1, B], f32)
        nsce = sb.tile([1, B], f32)
        nc.scalar.activation(nsce[:], nscp[:], Act.Exp)
        nc.vector.tensor_scalar(out=nsce[:], in0=nsce[:], scalar1=1.0,
                                scalar2=None, op0=Alu.add)
        nc.scalar.activation(nscT[:], nsce[:], Act.Ln)

        esc_b = transpose_pe(escT[:], 3, B, name="esc_b")     # [32, 3]
        row96 = sb.tile([1, 96], f32)
        nc.sync.dma_start(row96[:], esc_b[:])
        ns96 = sb.tile([1, 96], f32)
        nc.vector.tensor_copy(
            ns96[:].rearrange("o (b k) -> o b k", k=3),
            nscT[:].rearrange("o (b k) -> o b k", k=1).to_broadcast([1, B, 3]))

        # qn row patterns
        qn_flat = sb.tile([1, B * D], f32)
        nc.sync.dma_start(qn_flat[:], qre[:])
        qn2 = sb.tile([B, 1], f32)
        sqq = sb.tile([B, D], f32)
        nc.scalar.activation(sqq[:], qre[:], Act.Square, accum_out=qn2[:])
        qn_n = sb.tile([B, 1], f32)
        nc.scalar.activation(qn_n[:], qn2[:], Act.Sqrt)
        nc.vector.tensor_scalar(out=qn_n[:], in0=qn_n[:], scalar1=1e-8,
                                scalar2=None, op0=Alu.max)
        qn_row = transpose_pe(qn_n[:], B, 1, name="qn_row")   # [1, 32]
        qnorm256 = sb.tile([1, 256], f32)
        nc.vector.tensor_copy(
            qnorm256[:].rearrange("o (r b) -> o r b", b=B),
            qn_row[:].rearrange("o (r b) -> o r b", r=1).to_broadcast([1, 8, B]))

        QN = bcast_row(qn_flat[:], B * D, name="QN")          # [125, 2048]
        E96 = bcast_row(row96[:], 96, name="E96")
        NS96 = bcast_row(ns96[:], 96, name="NS96")
        QNR = bcast_row(qnorm256[:], 256, name="QNR")
        E1536 = sb.tile([125, 1536], f32)
        nc.vector.tensor_copy(
            E1536[:].rearrange("p (r f) -> p r f", r=16),
            E96[:].rearrange("p (r f) -> p r f", r=1).to_broadcast([125, 16, 96]))
        NS1536 = sb.tile([125, 1536], f32)
        nc.vector.tensor_copy(
            NS1536[:].rearrange("p (r f) -> p r f", r=16),
            NS96[:].rearrange("p (r f) -> p r f", r=1).to_broadcast([125, 16, 96]))
        Kb = bcast_col(ksb[:], parts=125, name="Kb")
        Kb128p = ps.tile([128, 1], f32, tag="pp")
        nc.tensor.matmul(out=Kb128p[:], lhsT=ones_1_128[:], rhs=ksb[:],
                         start=True, stop=True)
        Kb128 = sb.tile([128, 1], f32)
        nc.vector.tensor_copy(Kb128[:], Kb128p[:])

        # ================= stage 1: noise stats =================
        nz = st.tile([125, 512 * 3], f32, tag="nzA", bufs=2, name="nz")
        nc.sync.dma_start(nz[:], noise_sh.ap().rearrange("(p f) k -> p (f k)", p=125))
        nz3 = nz[:].rearrange("p (f k) -> p f k", k=3)
        esc_full = st.tile([125, 512 * 3], f32, tag="nzB", bufs=2, name="esc_full")
        ef3 = esc_full[:].rearrange("p (f k) -> p f k", k=3)
        # esc = noise * NS + E   (row patterns repeat every 96)
        NSfull = NS1536[:].rearrange("p (r f) -> p r f", f=96).to_broadcast([125, 16, 96])
        Efull = E1536[:].rearrange("p (r f) -> p r f", f=96).to_broadcast([125, 16, 96])
        nc.vector.tensor_tensor(out=esc_full[:].rearrange("p (r f) -> p r f", f=96),
                                in0=nz[:].rearrange("p (r f) -> p r f", f=96),
                                in1=NSfull, op=Alu.mult)
        nc.vector.tensor_tensor(out=esc_full[:].rearrange("p (r f) -> p r f", f=96),
                                in0=esc_full[:].rearrange("p (r f) -> p r f", f=96),
                                in1=Efull, op=Alu.add)
        mx3 = st.tile([125, 512], f32, tag="nz512", bufs=1, name="mx3")
        nc.vector.reduce_max(mx3[:], ef3, axis=Axis.X)
        esub = st.tile([125, 512 * 3], f32, tag="nzA", bufs=2, name="esub")
        nc.vector.tensor_tensor(out=esub[:].rearrange("p (f k) -> p f k", k=3),
                                in0=ef3, in1=mx3[:].to_broadcast([125, 512, 3]),
                                op=Alu.subtract)
        eexp = st.tile([125, 512 * 3], f32, tag="nzB", bufs=2, name="eexp")
        nc.scalar.activation(eexp[:], esub[:], Act.Exp)
        ssum = st.tile([125, 512], f32, tag="nz512b", bufs=1, name="ssum")
        nc.vector.reduce_sum(ssum[:], eexp[:].rearrange("p (f k) -> p f k", k=3),
                             axis=Axis.X)
        rsum = st.tile([125, 512], f32, tag="nz512", bufs=1, name="rsum")
        nc.vector.reciprocal(rsum[:], ssum[:])
        wgt = st.tile([125, 512 * 3], f32, tag="nzA", bufs=2, name="wgt")
        nc.vector.tensor_tensor(out=wgt[:].rearrange("p (f k) -> p f k", k=3),
                                in0=eexp[:].rearrange("p (f k) -> p f k", k=3),
                                in1=rsum[:].to_broadcast([125, 512, 3]), op=Alu.mult)
        sk_part = sb.tile([125, 3], f32)
        nc.vector.reduce_sum(sk_part[:], wgt[:].rearrange("p (f k) -> p k f", k=3),
                             axis=Axis.X)
        wsq = st.tile([125, 512 * 3], f32, tag="nzB", bufs=2, name="wsq")
        nc.scalar.activation(wsq[:], wgt[:], Act.Square)
        sq_part = sb.tile([125, 1], f32)
        nc.vector.reduce_sum(sq_part[:], wsq[:], axis=Axis.X)
        skp = ps.tile([3, 1], f32, tag="pp")
        nc.tensor.matmul(out=skp[:], lhsT=sk_part[:], rhs=ones125_1[:],
                         start=True, stop=True)
        sqp = ps.tile([1, 1], f32, tag="pp")
        nc.tensor.matmul(out=sqp[:], lhsT=sq_part[:], rhs=ones125_1[:],
                         start=True, stop=True)
        sk_sb = sb.tile([3, 1], f32)
        nc.vector.tensor_copy(sk_sb[:], skp[:])
        sk_row = transpose_pe(sk_sb[:], 3, 1, name="sk_row")  # [1, 3]
        arin = dr.tile([1, 8], f32)
        arout = dr.tile([1, 8], f32)
        ar_sb = sb.tile([1, 8], f32)
        nc.vector.memset(ar_sb[:], 0.0)
        nc.vector.tensor_copy(ar_sb[:, 0:3], sk_row[:])
        nc.vector.tensor_copy(ar_sb[:, 3:4], sqp[:])
        nc.sync.dma_start(arin[:], ar_sb[:])
        nc.gpsimd.collective_compute(
            "AllReduce", Alu.add, replica_groups=[list(range(NCORES))],
            ins=[arin.opt()], outs=[arout.opt()])
        ar = sb.tile([1, 8], f32)
        nc.sync.dma_start(ar[:], arout[:])

        # importance / L_imp
        stot = sb.tile([1, 1], f32)
        nc.vector.reduce_sum(stot[:], ar[:, 0:3], axis=Axis.X)
        rstot = sb.tile([1, 1], f32)
        nc.vector.reciprocal(rstot[:], stot[:])
        w3 = sb.tile([1, 3], f32)
        nc.vector.tensor_tensor(out=w3[:], in0=ar[:, 0:3],
                                in1=rstot[:].to_broadcast([1, 3]), op=Alu.mult)
        wsum = sb.tile([1, 1], f32)
        nc.vector.reduce_sum(wsum[:], w3[:], axis=Axis.X)
        n3 = float(3 * N)
        mu = sb.tile([1, 1], f32)
        nc.vector.tensor_scalar(out=mu[:], in0=stot[:], scalar1=1.0 / n3,
                                scalar2=None, op0=Alu.mult)
        musq = sb.tile([1, 1], f32)
        nc.vector.tensor_tensor(out=musq[:], in0=mu[:], in1=mu[:], op=Alu.mult)
        var = sb.tile([1, 1], f32)
        nc.vector.tensor_scalar(out=var[:], in0=musq[:], scalar1=-n3,
                                scalar2=None, op0=Alu.mult)
        nc.vector.tensor_tensor(out=var[:], in0=var[:], in1=ar[:, 3:4], op=Alu.add)
        nc.vector.tensor_scalar(out=var[:], in0=var[:], scalar1=1.0 / (n3 - 1.0),
                                scalar2=None, op0=Alu.mult)
        mue = sb.tile([1, 1], f32)
        nc.vector.tensor_scalar(out=mue[:], in0=mu[:], scalar1=1e-5,
                                scalar2=None, op0=Alu.add)
        mue2 = sb.tile([1, 1], f32)
        nc.vector.tensor_tensor(out=mue2[:], in0=mue[:], in1=mue[:], op=Alu.mult)
        mue2r = sb.tile([1, 1], f32)
        nc.vector.reciprocal(mue2r[:], mue2[:])
        limp_sb = sb.tile([1, 1], f32)
        nc.vector.tensor_tensor(out=limp_sb[:], in0=var[:], in1=mue2r[:],
                                op=Alu.mult)
        nc.sync.dma_start(limp.ap(), limp_sb[:])

        w0b = bcast_col(w3[:, 0:1], name="w0b")
        w1b = bcast_col(w3[:, 1:2], name="w1b")
        w2b = bcast_col(w3[:, 2:3], name="w2b")
        wsb_ = bcast_col(wsum[:], name="wsb_")

        # ================= stage 2: cos over own diff block =================
        dots = sb.tile([125, 256], f32)
        nrm2 = sb.tile([125, 256], f32)
        for j in range(8):
            hdj = st.tile([125, 32 * D], f32, tag="stA")
            nc.sync.dma_start(
                hdj[:], hid_df.ap().rearrange("(p j i) d -> p j (i d)", p=125, j=8)[:, j])
            prod = st.tile([125, 32 * D], f32, tag="stB", bufs=2)
            nc.vector.tensor_tensor(out=prod[:], in0=hdj[:], in1=QN[:], op=Alu.mult)
            nc.vector.reduce_sum(dots[:, 32 * j:32 * (j + 1)],
                                 prod[:].rearrange("p (i d) -> p i d", d=D),
                                 axis=Axis.X)
            sqh = st.tile([125, 32 * D], f32, tag="stB", bufs=2)
            nc.scalar.activation(sqh[:], hdj[:], Act.Square)
            nc.vector.reduce_sum(nrm2[:, 32 * j:32 * (j + 1)],
                                 sqh[:].rearrange("p (i d) -> p i d", d=D),
                                 axis=Axis.X)
        hn = st.tile([125, 256], f32, tag="cosA", bufs=1, name="hn")
        nc.scalar.activation(hn[:], nrm2[:], Act.Sqrt)
        nc.vector.tensor_scalar(out=hn[:], in0=hn[:], scalar1=1e-8,
                                scalar2=None, op0=Alu.max)
        den = st.tile([125, 256], f32, tag="cosB", bufs=1, name="den")
        nc.vector.tensor_tensor(out=den[:], in0=hn[:], in1=QNR[:], op=Alu.mult)
        rden = st.tile([125, 256], f32, tag="cosA", bufs=1, name="rden")
        nc.vector.reciprocal(rden[:], den[:])
        cos_own = sb.tile([125, 256], f32)
        nc.vector.tensor_tensor(out=cos_own[:], in0=dots[:], in1=rden[:],
                                op=Alu.mult)

        # ================= stage 3: local edge candidates =================
        alph = st.tile([125, EP], f32, tag="edgeA", bufs=1, name="alph")
        nc.sync.dma_start(alph[:], alpha_sh.ap().rearrange("(p e) o -> p (e o)", p=125))
        objt = st.tile([125, EP], f32, tag="edgeB", bufs=1, name="objt")
        nc.sync.dma_start(objt[:], obj_sh.ap().rearrange("(p e) o -> p (e o)", p=125))
        dmask = st.tile([125, EP], dt.int32, tag="edgeC", bufs=1, name="dmask")
        nc.vector.tensor_scalar(out=dmask[:], in0=objt[:], scalar1=float(NDIFF),
                                scalar2=None, op0=Alu.is_ge)
        negt = st.tile([125, EP], f32, tag="edgeD", bufs=1, name="negt")
        nc.vector.memset(negt[:], NEG)
        wk = [st.tile([125, EP], f32, tag="wkt" + str(i % 2), bufs=1, name=f"wk{i}") for i in range(4)]
        nc.vector.select(wk[0][:], dmask[:], alph[:], negt[:])
        cvals = sb.tile([125, CAND], f32)
        cpos = sb.tile([125, CAND], dt.uint32)
        for r in range(3):
            v8 = cvals[:, 8 * r:8 * (r + 1)]
            nc.vector.max(v8, wk[r][:])
            nc.vector.max_index(cpos[:, 8 * r:8 * (r + 1)], v8, wk[r][:])
            nc.vector.match_replace(wk[r + 1][:], v8, wk[r][:], SENT)
        # candidate objs: gather obj_sh[P*1000 + pos]
        pbase = sb.tile([125, CAND], dt.int32)
        nc.gpsimd.iota(pbase[:], pattern=[[0, CAND]], base=0, channel_multiplier=EP)
        cposi = sb.tile([125, CAND], dt.int32)
        nc.vector.tensor_copy(cposi[:], cpos[:])
        cflat = sb.tile([125, CAND], dt.int32)
        nc.vector.tensor_tensor(out=cflat[:], in0=pbase[:], in1=cposi[:], op=Alu.add)
        cobj = sb.tile([125, CAND], f32)
        for i in range(CAND):
            nc.gpsimd.indirect_dma_start(
                out=cobj[:, i:i + 1], out_offset=None, in_=obj_sh[:],
                in_offset=bass.IndirectOffsetOnAxis(ap=cflat[:, i:i + 1], axis=0))

        # ================= stage 4: slab + AllGather =================
        slab = dr.tile([1, SLAB], f32)
        gath = dr.tile([NCORES, SLAB], f32)
        nc.sync.dma_start(
            slab[0:1, COS_OFF:COS_OFF + HBLK].rearrange("o (p u) -> p (o u)", p=125),
            cos_own[:])
        padv = sb.tile([125, CPAD], f32)
        nc.vector.memset(padv[:], NEG)
        nc.vector.tensor_copy(padv[:, 0:CAND], cvals[:])
        nc.sync.dma_start(
            slab[0:1, VAL_OFF:VAL_OFF + 125 * CPAD].rearrange("o (p u) -> p (o u)", p=125),
            padv[:])
        pado = sb.tile([125, CPAD], f32)
        nc.vector.memset(pado[:], 0.0)
        nc.vector.tensor_copy(pado[:, 0:CAND], cobj[:])
        nc.sync.dma_start(
            slab[0:1, OBJ_OFF:OBJ_OFF + 125 * CPAD].rearrange("o (p u) -> p (o u)", p=125),
            pado[:])
        nc.gpsimd.collective_compute(
            "AllGather", Alu.bypass, replica_groups=[list(range(NCORES))],
            ins=[slab.opt()], outs=[gath.opt()])

        # ============ stage 5: binary searches (replicated) ============
        def count_search(data_ap, parts, merge_lhsT, kvec, lo0, hi0, name):
            """Exact top-K threshold: returns lo tile [parts,1] with
            count(v >= lo) == K per group. data [parts, F]."""
            lo = st.tile([parts, 1], f32, tag=name + "_lo", bufs=2, name=name + "_lo0")
            hi = st.tile([parts, 1], f32, tag=name + "_hi", bufs=2, name=name + "_hi0")
            tt = st.tile([parts, 1], f32, tag=name + "_tt", bufs=2, name=name + "_tt0")
            nc.vector.memset(lo[:], lo0)
            nc.vector.memset(hi[:], hi0)
            nc.vector.memset(tt[:], (lo0 + hi0) / 2.0)
            F = data_ap.shape[-1]
            nch = (F + 511) // 512
            FP = nch * 512
            cmp = st.tile([parts, FP], f32, tag="cmpbig" if FP > 512 else "cmpsm", bufs=1, name=name + "_cmp")
            nc.vector.memset(cmp[:], 0.0)
            for it in range(SEARCH_ITERS):
                nc.vector.tensor_scalar(out=cmp[:, :F], in0=data_ap, scalar1=tt[:],
                                        scalar2=None, op0=Alu.is_ge)
                mg = ps.tile([parts, 512], f32, tag="mg")
                for ch in range(nch):
                    nc.tensor.matmul(out=mg[:], lhsT=merge_lhsT,
                                     rhs=cmp[:, 512 * ch:512 * (ch + 1)],
                                     start=(ch == 0), stop=(ch == nch - 1))
                cnt = st.tile([parts, 1], f32, tag="cnt", bufs=2)
                nc.vector.reduce_sum(cnt[:], mg[:], axis=Axis.X)
                ge = st.tile([parts, 1], dt.int32, tag="geb", bufs=2)
                nc.vector.tensor_tensor(out=ge[:], in0=cnt[:], in1=kvec, op=Alu.is_ge)
                lo2 = st.tile([parts, 1], f32, tag=name + "_lo", bufs=2, name=f"{name}_lo{it + 1}")
                hi2 = st.tile([parts, 1], f32, tag=name + "_hi", bufs=2, name=f"{name}_hi{it + 1}")
                nc.vector.select(lo2[:], ge[:], tt[:], lo[:])
                nc.vector.select(hi2[:], ge[:], hi[:], tt[:])
                lo, hi = lo2, hi2
                if it < SEARCH_ITERS - 1:
                    tt = st.tile([parts, 1], f32, tag=name + "_tt", bufs=2, name=f"{name}_tt{it + 1}")
                    nc.vector.tensor_tensor(out=tt[:], in0=lo[:], in1=hi[:], op=Alu.add)
                    nc.vector.tensor_scalar(out=tt[:], in0=tt[:], scalar1=0.5,
                                            scalar2=None, op0=Alu.mult)
            return lo

        def load_bq(src_dram_rearr, name):
            """Load [125, 2048] then 16 PE-transposes -> [128, 2000] (b,q)."""
            flat = st.tile([125, 2048], f32, tag="flat", bufs=2, name=name + "_f")
            nc.sync.dma_start(flat[:], src_dram_rearr)
            out = sb.tile([128, 2000], f32, name=name + "_t")
            for k in range(16):
                p = ps.tile([128, 125], f32, tag="pp")
                nc.tensor.transpose(p[:], flat[:, 128 * k:128 * (k + 1)],
                                    id128[:125, :125])
                nc.vector.tensor_copy(out[:, 125 * k:125 * (k + 1)], p[:])
            return out

        # 5a: scores
        SC = load_bq(sc_diff_full.ap().rearrange("(p f) o -> p (f o)", p=125), "sc")
        t0 = count_search(SC[:], 128, M128[:], Kb128[:], 0.0, 1.2, "s0")
        # 5b: cos (from gathered slabs)
        cosf = st.tile([125, 2048], f32, tag="flat", bufs=2, name="cosf")
        for q in range(NCORES):
            nc.sync.dma_start(
                cosf[:, 256 * q:256 * (q + 1)],
                gath[q:q + 1, COS_OFF:COS_OFF + HBLK].rearrange(
                    "o (p u) -> p (o u)", p=125))
        # need node-order within free: element (p, q, u) -> b = u % 32 OK
        CS = sb.tile([128, 2000], f32)
        for k in range(16):
            p = ps.tile([128, 125], f32, tag="pp")
            nc.tensor.transpose(p[:], cosf[:, 128 * k:128 * (k + 1)],
                                id128[:125, :125])
            nc.vector.tensor_copy(CS[:, 125 * k:125 * (k + 1)], p[:])
        t1 = count_search(CS[:], 128, M128[:], Kb128[:], -1.01, 1.01, "s1")

        # 5c: expert-2 global
        candv = sb.tile([125, NCORES * CPAD], f32)
        cando = sb.tile([125, NCORES * CPAD], f32)
        for q in range(NCORES):
            nc.sync.dma_start(
                candv[:, CPAD * q:CPAD * (q + 1)],
                gath[q:q + 1, VAL_OFF:VAL_OFF + 125 * CPAD].rearrange(
                    "o (p u) -> p (o u)", p=125))
            nc.sync.dma_start(
                cando[:, CPAD * q:CPAD * (q + 1)],
                gath[q:q + 1, OBJ_OFF:OBJ_OFF + 125 * CPAD].rearrange(
                    "o (p u) -> p (o u)", p=125))
        gwk = [sb.tile([125, NCORES * CPAD], f32, name=f"gwk{i}") for i in range(3)]
        nc.vector.tensor_copy(gwk[0][:], candv[:])
        v16 = sb.tile([125, 16], f32)
        p16 = sb.tile([125, 16], dt.uint32)
        for r in range(2):
            v8 = v16[:, 8 * r:8 * (r + 1)]
            nc.vector.max(v8, gwk[r][:])
            nc.vector.max_index(p16[:, 8 * r:8 * (r + 1)], v8, gwk[r][:])
            nc.vector.match_replace(gwk[r + 1][:], v8, gwk[r][:], SENT)
        # obj lookup: pos -> q = pos >> 5, r = pos & 31
        p16i = sb.tile([125, 16], dt.int32)
        nc.vector.tensor_copy(p16i[:], p16[:])
        qq = sb.tile([125, 16], dt.int32)
        nc.vector.tensor_scalar(out=qq[:], in0=p16i[:], scalar1=5, scalar2=None,
                                op0=Alu.logical_shift_right)
        rr = sb.tile([125, 16], dt.int32)
        nc.vector.tensor_scalar(out=rr[:], in0=p16i[:], scalar1=31, scalar2=None,
                                op0=Alu.bitwise_and)
        goff = sb.tile([125, 16], dt.int32)
        nc.vector.tensor_scalar(out=goff[:], in0=qq[:], scalar1=SLAB,
                                scalar2=None, op0=Alu.mult)
        nc.vector.tensor_tensor(out=goff[:], in0=goff[:], in1=rr[:], op=Alu.add)
        pb16 = sb.tile([125, 16], dt.int32)
        nc.gpsimd.iota(pb16[:], pattern=[[0, 16]], base=OBJ_OFF,
                       channel_multiplier=CPAD)
        nc.vector.tensor_tensor(out=goff[:], in0=goff[:], in1=pb16[:], op=Alu.add)
        o16 = sb.tile([125, 16], f32)
        gath_flat = gath[:, :].rearrange("a (b o) -> (a b) o", o=1)
        for i in range(16):
            nc.gpsimd.indirect_dma_start(
                out=o16[:, i:i + 1], out_offset=None, in_=gath_flat,
                in_offset=bass.IndirectOffsetOnAxis(ap=goff[:, i:i + 1], axis=0))
        # rep test via scatter-add table
        TBLN = 262144
        addtab = dr.tile([TBLN, 1], f32)
        m2tab = dr.tile([TBLN, 1], f32)
        zt = st.tile([128, 2048], f32, tag="ztg", bufs=1, name="zt")
        nc.vector.memset(zt[:], 0.0)
        nc.sync.dma_start(addtab[:, :].rearrange("(p f) o -> p (f o)", p=128), zt[:])
        nc.sync.dma_start(m2tab[:, :].rearrange("(p f) o -> p (f o)", p=128), zt[:])
        idx16 = sb.tile([125, 16], dt.int32)
        o16s = sb.tile([125, 16], f32)
        nc.vector.tensor_scalar(out=o16s[:], in0=o16[:], scalar1=float(NDIFF),
                                scalar2=None, op0=Alu.subtract)
        nc.vector.tensor_copy(idx16[:], o16s[:])
        for i in range(16):
            nc.gpsimd.indirect_dma_start(
                out=addtab[:], out_offset=bass.IndirectOffsetOnAxis(
                    ap=idx16[:, i:i + 1], axis=0),
                in_=v16[:, i:i + 1], in_offset=None, compute_op=Alu.add)
        gval = sb.tile([125, 16], f32)
        for i in range(16):
            nc.gpsimd.indirect_dma_start(
                out=gval[:, i:i + 1], out_offset=None, in_=addtab[:],
                in_offset=bass.IndirectOffsetOnAxis(ap=idx16[:, i:i + 1], axis=0))
        rep = sb.tile([125, 16], dt.int32)
        nc.vector.tensor_tensor(out=rep[:], in0=gval[:], in1=v16[:], op=Alu.is_equal)
        neg100 = sb.tile([125, 16], f32)
        nc.vector.memset(neg100[:], -100.0)
        repv = sb.tile([125, 16], f32)
        nc.vector.select(repv[:], rep[:], v16[:], neg100[:])
        t2 = count_search(repv[:], 125, ones125sq[:], Kb[:], -50.0, 50.0, "s2")
        wmask = sb.tile([125, 16], f32)
        nc.vector.tensor_scalar(out=wmask[:], in0=repv[:], scalar1=t2[:],
                                scalar2=None, op0=Alu.is_ge)
        bigoff = sb.tile([125, 16], dt.int32)
        nc.vector.memset(bigoff[:], 99999999)
        woff = sb.tile([125, 16], dt.int32)
        wmaski = sb.tile([125, 16], dt.int32)
        nc.vector.tensor_copy(wmaski[:], wmask[:])
        nc.vector.select(woff[:], wmaski[:], idx16[:], bigoff[:])
        for i in range(16):
            nc.gpsimd.indirect_dma_start(
                out=m2tab[:], out_offset=bass.IndirectOffsetOnAxis(
                    ap=woff[:, i:i + 1], axis=0),
                in_=onecol[:, 0:1], in_offset=None,
                bounds_check=TBLN - 1, oob_is_err=False)

        # ================= stage 6: wm + outputs =================
        # t0/t1 rows -> [125, 256] patterns
        def thr_pattern(tvec, name):
            trow = transpose_pe(tvec, 128, 1, name=name + "_r")  # [1,128]
            t256 = sb.tile([1, 256], f32, name=name + "_256")
            nc.vector.tensor_copy(
                t256[:].rearrange("o (r b) -> o r b", b=B),
                trow[:, 0:B].rearrange("o (r b) -> o r b", r=1).to_broadcast([1, 8, B]))
            return bcast_row(t256[:], 256, name=name + "_p")

        T0 = thr_pattern(t0[:], "T0")
        T1 = thr_pattern(t1[:], "T1")

        # m2 own slice via indirect gather of [1024, 256] rows
        iot = sb.tile([125, 1], dt.int32)
        nc.gpsimd.iota(iot[:], pattern=[[0, 1]], base=0, channel_multiplier=1)
        cb125f = bcast_col(cbsb[:], name="cb125f")
        cb125 = sb.tile([125, 1], dt.int32)
        nc.vector.tensor_copy(cb125[:], cb125f[:])
        nc.vector.tensor_scalar(out=cb125[:], in0=cb125[:], scalar1=125,
                                scalar2=None, op0=Alu.mult)
        nc.vector.tensor_tensor(out=cb125[:], in0=cb125[:], in1=iot[:], op=Alu.add)
        m2own = sb.tile([125, 256], f32)
        nc.gpsimd.indirect_dma_start(
            out=m2own[:], out_offset=None,
            in_=m2tab[:, :].rearrange("(r u) o -> r (u o)", u=256),
            in_offset=bass.IndirectOffsetOnAxis(ap=cb125[:, 0:1], axis=0))

        scown = sb.tile([125, 256], f32)
        nc.sync.dma_start(scown[:], sc_df.ap().rearrange("(p u) o -> p (u o)", p=125))
        m0 = st.tile([125, 256], f32, tag="wmA", bufs=1, name="m0t")
        nc.vector.tensor_tensor(out=m0[:], in0=scown[:], in1=T0[:], op=Alu.is_ge)
        m1 = st.tile([125, 256], f32, tag="wmB", bufs=1, name="m1t")
        nc.vector.tensor_tensor(out=m1[:], in0=cos_own[:], in1=T1[:], op=Alu.is_ge)
        wm = sb.tile([125, 256], f32)
        nc.vector.tensor_scalar(out=wm[:], in0=m0[:], scalar1=w0b[:],
                                scalar2=None, op0=Alu.mult)
        tmp1 = st.tile([125, 256], f32, tag="wmA", bufs=1, name="tmp1")
        nc.vector.tensor_scalar(out=tmp1[:], in0=m1[:], scalar1=w1b[:],
                                scalar2=None, op0=Alu.mult)
        nc.vector.tensor_tensor(out=wm[:], in0=wm[:], in1=tmp1[:], op=Alu.add)
        tmp2 = st.tile([125, 256], f32, tag="wmB", bufs=1, name="tmp2")
        nc.vector.tensor_scalar(out=tmp2[:], in0=m2own[:], scalar1=w2b[:],
                                scalar2=None, op0=Alu.mult)
        nc.vector.tensor_tensor(out=wm[:], in0=wm[:], in1=tmp2[:], op=Alu.add)

        if debug:
            dbt = sb.tile([128, 4], f32)
            nc.vector.memset(dbt[:], 0.0)
            nc.vector.tensor_copy(dbt[:, 0:1], t0[:])
            nc.vector.tensor_copy(dbt[:, 1:2], t1[:])
            nc.vector.tensor_copy(dbt[:125, 2:3], t2[:])
            nc.sync.dma_start(dbg_t.ap(), dbt[:])
            nc.sync.dma_start(dbg_w.ap(), ar[:])
            nc.sync.dma_start(dbg_wm.ap(), wm[:])
            nc.sync.dma_start(dbg_cos.ap(), cos_own[:])

        # scores out
        sccd = st.tile([125, 256], f32, tag="wmA", bufs=1, name="sccd")
        nc.vector.tensor_tensor(out=sccd[:], in0=scown[:], in1=wm[:], op=Alu.mult)
        nc.sync.dma_start(scc_df.ap().rearrange("(p u) o -> p (u o)", p=125), sccd[:])
        scnd = st.tile([125, 256], f32, tag="wmB", bufs=1, name="scnd")
        nc.sync.dma_start(scnd[:], sc_nd.ap().rearrange("(p u) o -> p (u o)", p=125))
        sccn = st.tile([125, 256], f32, tag="wmA", bufs=1, name="sccn")
        nc.vector.tensor_scalar(out=sccn[:], in0=scnd[:], scalar1=wsb_[:],
                                scalar2=None, op0=Alu.mult)
        nc.sync.dma_start(scc_nd.ap().rearrange("(p u) o -> p (u o)", p=125), sccn[:])

        # big streams
        def scale_const(src, dst, eng_alt):
            for j in range(8):
                tl = st.tile([125, 32 * D], f32, tag="stA")
                nc.sync.dma_start(
                    tl[:], src.ap().rearrange("(p j i) d -> p j (i d)", p=125, j=8)[:, j])
                ot = st.tile([125, 32 * D], f32, tag="stB", bufs=2)
                eng = nc.gpsimd if (eng_alt and j % 2 == 0) else nc.vector
                eng.tensor_scalar(out=ot[:], in0=tl[:], scalar1=wsb_[:],
                                  scalar2=None, op0=Alu.mult)
                nc.sync.dma_start(
                    dst.ap().rearrange("(p j i) d -> p j (i d)", p=125, j=8)[:, j], ot[:])

        def scale_wm(src, dst):
            for j in range(8):
                tl = st.tile([125, 32 * D], f32, tag="stA")
                nc.sync.dma_start(
                    tl[:], src.ap().rearrange("(p j i) d -> p j (i d)", p=125, j=8)[:, j])
                ot = st.tile([125, 32 * D], f32, tag="stB", bufs=2)
                eng = nc.gpsimd if j % 2 == 0 else nc.vector
                eng.tensor_tensor(
                    out=ot[:].rearrange("p (i d) -> p i d", d=D),
                    in0=tl[:].rearrange("p (i d) -> p i d", d=D),
                    in1=wm[:, 32 * j:32 * (j + 1)].to_broadcast([125, 32, D]),
                    op=Alu.mult)
                nc.sync.dma_start(
                    dst.ap().rearrange("(p j i) d -> p j (i d)", p=125, j=8)[:, j], ot[:])

        scale_const(hid_nd, hidc_nd, True)
        scale_const(h0_nd, h0c_nd, True)
        scale_wm(hid_df, hidc_df)
        scale_wm(h0_df, h0c_df)


    nc.compile()
    return nc


_NC_CACHE = {}


def _get_nc(debug=False):
    if debug not in _NC_CACHE:
        _NC_CACHE[debug] = build_kernel(debug=debug)
    return _NC_CACHE[debug]


def kernel(hidden, nodes, scores, h0, alpha, hidden_q, q_rel, edges,
           old_nodes_new_idx, batch_size, message, obj, alpha_temp, l, noise,
           mlp_w1, mlp_b1, mlp_w2, mlp_b2, rel_embed, expert_emb, w_n,
           _debug=False, _trace=False):
    nc = _get_nc(debug=_debug)
    f = np.float32
    K_l = float(compute_topk(int(l)))
    hidden = np.ascontiguousarray(hidden, f)
    h0v = np.ascontiguousarray(np.asarray(h0, f).reshape(N, D))
    scores = np.ascontiguousarray(scores, f)
    noise = np.ascontiguousarray(noise, f)
    alpha_t = np.ascontiguousarray(alpha_temp, f).reshape(NEDGE, 1)
    obj_e = np.ascontiguousarray(np.asarray(edges)[:, 5].astype(f)).reshape(NEDGE, 1)
    sc_diff_full = np.ascontiguousarray(scores[NDIFF:], f).reshape(NDIFF, 1)
    shared = {
        "sc_diff_full": sc_diff_full,
        "hidden_q": np.ascontiguousarray(hidden_q, f),
        "q_rel": np.ascontiguousarray(np.asarray(q_rel).astype(np.int32)).reshape(B, 1),
        "rel_embed": np.ascontiguousarray(rel_embed, f),
        "mlp_w1": np.ascontiguousarray(mlp_w1, f),
        "mlp_b1": np.ascontiguousarray(mlp_b1, f).reshape(D, 1),
        "mlp_w2": np.ascontiguousarray(mlp_w2, f),
        "mlp_b2": np.ascontiguousarray(mlp_b2, f).reshape(D, 1),
        "expert_emb": np.ascontiguousarray(expert_emb, f),
        "w_n": np.ascontiguousarray(w_n, f).reshape(D, 1),
        "k_in": np.array([[K_l]], f),
    }
    in_maps = []
    for c in range(NCORES):
        a, b = HBLK * c, HBLK * (c + 1)
        da, db = NDIFF + HBLK * c, NDIFF + HBLK * (c + 1)
        ea, eb = ESH * c, ESH * (c + 1)
        m = dict(shared)
        m.update({
            "hid_nd": hidden[a:b], "hid_df": hidden[da:db],
            "h0_nd": h0v[a:b], "h0_df": h0v[da:db],
            "sc_nd": scores[a:b].reshape(HBLK, 1),
            "sc_df": scores[da:db].reshape(HBLK, 1),
            "noise_sh": np.ascontiguousarray(
                np.concatenate([noise[a:b], noise[da:db]], axis=0)),
            "alpha_sh": alpha_t[ea:eb], "obj_sh": obj_e[ea:eb],
            "cbase": np.array([[float(c)]], f),
        })
        in_maps.append(m)

    res = bass_utils.run_bass_kernel_spmd(
        nc, in_maps, core_ids=list(range(NCORES)), trace=_trace)
    outs = res.results

    hidden_c = np.empty((N, D), f)
    scores_c = np.empty((N,), f)
    h0_c = np.empty((N, D), f)
    for c in range(NCORES):
        a, b = HBLK * c, HBLK * (c + 1)
        da, db = NDIFF + HBLK * c, NDIFF + HBLK * (c + 1)
        o = outs[c]
        hidden_c[a:b] = o["hidc_nd"]; hidden_c[da:db] = o["hidc_df"]
        h0_c[a:b] = o["h0c_nd"]; h0_c[da:db] = o["h0c_df"]
        scores_c[a:b] = o["scc_nd"][:, 0]; scores_c[da:db] = o["scc_df"][:, 0]
    L_imp = np.float32(outs[0]["limp"][0, 0])
    ret = (hidden_c, scores_c, h0_c[None], L_imp)
    if _debug:
        dbg = {k: outs[0][k] for k in ("dbg_t", "dbg_w", "dbg_wm", "dbg_cos")}
        return ret, dbg, res
    return ret
